# revision 1
# baseline (speedup 1.0000x reference)
"""CrossModalAttention Trainium2 kernel.

Per-core (data-parallel over batch B=8 -> 8 NeuronCores):
  y_b = softmax((x_b Wq)(x_b Wk)^T * SCALE * (1 + mask_b)) (x_b Wv) @ Wo + bo

Design (transposed-softmax layout; all transposes done on host):
  - Host precomputes xT = x^T and m1T = (1+mask)^T per batch element, so
    the kernel never transposes on-chip: mask tiles DMA in naturally as
    f32 rows, SCALE is folded into Wq.
  - Scores are computed transposed, sT[j, i], two heads per PE pass via
    row tiling at partitions 0/64 (the second tile executes concurrently).
  - Softmax runs without max-subtraction (|t| <= ~11, exp safe in fp32):
    DVE multiplies sT(PSUM) by m1T, ACT exponentiates into f32r, and the
    AV matmul uses a ones-augmented V (65th column) so the softmax
    denominators fall out of the same accumulation for free (row 64).
  - Score matmuls are issued two j-tiles ahead of the AV matmuls so the
    PE never stalls on the DVE/ACT elementwise pipeline.
  - Normalization is deferred per i-chunk: denominator rows are stashed
    into one [8, 512] tile, a single batched DVE reciprocal replaces 8
    single-partition ones, PE outer-products broadcast the scales.
  - y = outT^T-contracted matmul against Wo with bias folded in via a
    K=1 ones x bo outer product; y stores DMA straight out of PSUM.
"""

import numpy as np

B, N, D = 8, 2048, 512
H, DH = 8, 64
SCALE = DH ** -0.5

IC_N, IC = 4, 512      # i-chunks
JT_N, JT = 16, 128     # j-tiles
P_N = 4                # head pairs

_built = {}


def _build():
    import concourse.tile as tile
    from concourse import bacc, mybir

    F32 = mybir.dt.float32
    F32R = mybir.dt.float32r
    Exp = mybir.ActivationFunctionType.Exp
    MULT = mybir.AluOpType.mult

    nc = bacc.Bacc()
    xT_d = nc.declare_dram_parameter("xT", [D, N], F32R, isOutput=False)
    m1T_d = nc.declare_dram_parameter("m1T", [N, N], F32, isOutput=False)
    wq_d = nc.declare_dram_parameter("wq", [D, D], F32R, isOutput=False)
    wk_d = nc.declare_dram_parameter("wk", [D, D], F32R, isOutput=False)
    wv_d = nc.declare_dram_parameter("wv", [D, D], F32R, isOutput=False)
    wo_d = nc.declare_dram_parameter("wo", [D, D], F32R, isOutput=False)
    bo_d = nc.declare_dram_parameter("bo", [1, D], F32R, isOutput=False)
    y_d = nc.declare_dram_parameter("y", [N, D], F32, isOutput=True)

    with nc.allow_low_precision(reason="f32r matmul pipeline"), \
         tile.TileContext(nc) as tc:
        with tc.tile_pool(name="persist", bufs=1) as pp:
            ones_f = pp.tile([128, 128], F32, tag="ones_f")
            nc.vector.memset(ones_f, 1.0)
            ones_r = pp.tile([128, 128], F32R, tag="ones_r")
            nc.vector.tensor_copy(ones_r[:], ones_f[:])

            qT = [pp.tile([128, N], F32R, tag=f"qT{t}", name=f"qT{t}") for t in range(4)]
            kT = [pp.tile([128, N], F32R, tag=f"kT{t}", name=f"kT{t}") for t in range(4)]
            v_sb = [pp.tile([128, H * 65], F32R, tag=f"v{t}", name=f"v{t}")
                    for t in range(JT_N)]
            wo_sb = [pp.tile([128, D], F32R, tag=f"wo{c}", name=f"wo{c}") for c in range(4)]
            bo_sb = pp.tile([1, D], F32R, tag="bo", name="bo")
            for c in range(4):
                nc.gpsimd.dma_start(out=wo_sb[c], in_=wo_d[c * 128:(c + 1) * 128, :])
            nc.gpsimd.dma_start(out=bo_sb, in_=bo_d[:])

            # ---------------- phase 1: projections ----------------
            with tc.tile_pool(name="ph01", bufs=1) as p1, \
                 tc.tile_pool(name="ph01ps", bufs=1, space="PSUM") as p1p:
                wq_sb = [p1.tile([128, D], F32R, tag=f"wq{c}", name=f"wq{c}") for c in range(4)]
                wk_sb = [p1.tile([128, D], F32R, tag=f"wk{c}", name=f"wk{c}") for c in range(4)]
                wv_sb = [p1.tile([128, D], F32R, tag=f"wv{c}", name=f"wv{c}") for c in range(4)]
                xT = [p1.tile([128, N], F32R, tag=f"xT{c}", name=f"xT{c}") for c in range(4)]
                for c in range(4):
                    nc.gpsimd.dma_start(out=wq_sb[c], in_=wq_d[c * 128:(c + 1) * 128, :])
                    nc.gpsimd.dma_start(out=wk_sb[c], in_=wk_d[c * 128:(c + 1) * 128, :])
                    nc.sync.dma_start(out=xT[c], in_=xT_d[c * 128:(c + 1) * 128, :])
                    nc.gpsimd.dma_start(out=wv_sb[c], in_=wv_d[c * 128:(c + 1) * 128, :])

                for w_sb, dstT, veng in ((wq_sb, qT, False), (wk_sb, kT, True)):
                    for hdt in range(4):
                        for nch in range(4):
                            qp = p1p.tile([128, 512], F32, tag="qp", bufs=2, name="qp")
                            for c in range(4):
                                nc.tensor.matmul(
                                    qp[:], w_sb[c][:, hdt * 128:(hdt + 1) * 128],
                                    xT[c][:, nch * 512:(nch + 1) * 512],
                                    start=(c == 0), stop=(c == 3))
                            dst = dstT[hdt][:, nch * 512:(nch + 1) * 512]
                            if veng:
                                nc.vector.tensor_copy(dst, qp[:])
                            else:
                                nc.scalar.copy(dst, qp[:])

                for nt in range(JT_N):
                    vp = p1p.tile([128, 512], F32, tag="vp", bufs=2, name="vp")
                    for c in range(4):
                        nc.tensor.matmul(vp[:], xT[c][:, nt * 128:(nt + 1) * 128],
                                         wv_sb[c][:], start=(c == 0), stop=(c == 3))
                    vdst = v_sb[nt].rearrange("p (h e) -> p h e", e=65)
                    nc.vector.tensor_copy(vdst[:, :, 0:64],
                                          vp[:].rearrange("p (h e) -> p h e", e=64))
                    nc.vector.tensor_copy(vdst[:, :, 64:65],
                                          ones_r[:, 0:H].rearrange("p (h e) -> p h e", e=1))

            # ---------------- phase 2: attention + per-ic y emission ----------------
            with tc.tile_pool(name="ph2", bufs=1) as p2, \
                 tc.tile_pool(name="ph2ps", bufs=1, space="PSUM") as p2p:
                for ic in range(IC_N):
                    m1T = []
                    for jt in range(JT_N):
                        m1t = p2.tile([128, IC], F32, tag="m1T", bufs=JT_N + 2, name="m1T")
                        nc.sync.dma_start(
                            out=m1t,
                            in_=m1T_d[jt * 128:(jt + 1) * 128, ic * IC:(ic + 1) * IC])
                        m1T.append(m1t)

                    outT = [p2.tile([128, IC], F32R, tag=f"oT{p}", name=f"oT{p}")
                            for p in range(P_N)]
                    pending_norm = None

                    for p in range(P_N):
                        av0 = p2p.tile([65, 512], F32, tag="av0", name="av0")
                        av1 = p2p.tile([65, 512], F32, tag="av1", name="av1")

                        e_gs = {}

                        def issue_scores(jt, p=p):
                            sp = p2p.tile([128, 1024], F32, tag="sp", bufs=2, name="sp")
                            nc.tensor.matmul(
                                sp[:, 0:512],
                                kT[p][0:64, jt * 128:(jt + 1) * 128],
                                qT[p][0:64, ic * IC:(ic + 1) * IC],
                                start=True, stop=True, tile_position=(0, 0))
                            nc.tensor.matmul(
                                sp[:, 512:1024],
                                kT[p][64:128, jt * 128:(jt + 1) * 128],
                                qT[p][64:128, ic * IC:(ic + 1) * IC],
                                start=True, stop=True, tile_position=(64, 0))
                            t_g = p2.tile([128, 1024], F32, tag="t_g", bufs=3, name="t_g")
                            nc.vector.tensor_tensor(
                                out=t_g[:].rearrange("p (h i) -> p h i", h=2),
                                in0=sp[:].rearrange("p (h i) -> p h i", h=2),
                                in1=m1T[jt][:, None, :].broadcast_to((128, 2, IC)),
                                op=MULT)
                            e_g = p2.tile([128, 1024], F32R, tag="e_g", bufs=3, name="e_g")
                            nc.scalar.activation(e_g[:], t_g[:], Exp)
                            e_gs[jt] = e_g

                        issue_scores(0)
                        issue_scores(1)
                        if pending_norm is not None:
                            pending_norm()
                            pending_norm = None
                        for jt in range(JT_N):
                            if jt + 2 < JT_N:
                                issue_scores(jt + 2)
                            e_g = e_gs.pop(jt)
                            for h in range(2):
                                hh = 2 * p + h
                                nc.tensor.matmul(
                                    (av0 if h == 0 else av1)[:],
                                    v_sb[jt][:, hh * 65:(hh + 1) * 65],
                                    e_g[:, h * 512:(h + 1) * 512],
                                    start=(jt == 0), stop=(jt == JT_N - 1))

                        def norm(p=p, av0=av0, av1=av1):
                            # denominators sit at PSUM row 64; reciprocals go to
                            # legal partition bases 64/32, then PE outer-products
                            # broadcast them over the 64 dh partitions.
                            rcp = p2.tile([128, 512], F32R, tag="rcp", bufs=2, name="rcp")
                            nc.vector.reciprocal(rcp[64:65, :], av0[64:65, :])
                            nc.vector.reciprocal(rcp[32:33, :], av1[64:65, :])
                            tmp = p2.tile([128, 512], F32, tag="tmp", bufs=2, name="tmp")
                            nc.scalar.copy(tmp[0:64, :], av0[0:64, :])
                            nc.scalar.copy(tmp[64:128, :], av1[0:64, :])
                            bc0 = p2p.tile([64, 512], F32, tag="bc0", name="bc0")
                            bc1 = p2p.tile([64, 512], F32, tag="bc1", name="bc1")
                            nc.tensor.matmul(bc0[:], ones_r[64:65, 0:64],
                                             rcp[64:65, :], start=True, stop=True)
                            nc.tensor.matmul(bc1[:], ones_r[32:33, 0:64],
                                             rcp[32:33, :], start=True, stop=True)
                            nc.vector.tensor_tensor(
                                out=outT[p][0:64, :], in0=tmp[0:64, :], in1=bc0[:],
                                op=MULT)
                            nc.vector.tensor_tensor(
                                out=outT[p][64:128, :], in0=tmp[64:128, :], in1=bc1[:],
                                op=MULT)

                        pending_norm = norm

                    pending_norm()
                    for itl in range(4):
                        it = ic * 4 + itl
                        ypt = p2p.tile([128, 1024], F32, tag="sp", bufs=2, name="yp")
                        yp = ypt[:, 0:512]
                        nc.tensor.matmul(yp, ones_r[0:1, 0:128], bo_sb[0:1, :],
                                         start=True, stop=False)
                        for hdt in range(4):
                            nc.tensor.matmul(yp, outT[hdt][:, itl * 128:(itl + 1) * 128],
                                             wo_sb[hdt][:], start=False, stop=(hdt == 3))
                        y_sb = p2.tile([128, D], F32, tag="y_sb", bufs=2, name="y_sb")
                        nc.scalar.copy(y_sb[:], yp)
                        nc.gpsimd.dma_start(out=y_d[it * 128:(it + 1) * 128, :], in_=y_sb[:])

    nc.finalize()
    return nc


def _get_nc():
    if "nc" not in _built:
        _built["nc"] = _build()
    return _built["nc"]


def _make_in_maps(inputs):
    x = np.asarray(inputs["x"], dtype=np.float32)
    mask = np.asarray(inputs["mask"], dtype=np.float32)
    wq = (np.asarray(inputs["Wq"], dtype=np.float32) * SCALE)
    wk = np.asarray(inputs["Wk"], dtype=np.float32)
    wv = np.asarray(inputs["Wv"], dtype=np.float32)
    wo = np.asarray(inputs["Wo"], dtype=np.float32)
    bo2 = np.asarray(inputs["bo"], dtype=np.float32).reshape(1, D)

    xT = np.ascontiguousarray(x.transpose(0, 2, 1))
    m1T = np.ascontiguousarray((1.0 + mask).transpose(0, 2, 1))

    return [
        {"xT": xT[b], "m1T": m1T[b],
         "wq": wq, "wk": wk, "wv": wv, "wo": wo, "bo": bo2}
        for b in range(B)
    ]


def kernel(x, mask, Wq, Wk, Wv, Wo, bo):
    from concourse.bass_utils import run_bass_kernel_spmd

    nc = _get_nc()
    in_maps = _make_in_maps(dict(x=x, mask=mask, Wq=Wq, Wk=Wk, Wv=Wv, Wo=Wo, bo=bo))
    res = run_bass_kernel_spmd(nc, in_maps, list(range(B)))
    return np.stack([res.results[b]["y"] for b in range(B)], axis=0)



# revision 17
# speedup vs baseline: 1.1776x; 1.1776x over previous
"""CrossModalAttention Trainium2 kernel (v6).

Per-core (data-parallel over batch B=8 -> 8 NeuronCores):
  y_b = softmax((x_b Wq)(x_b Wk)^T * SCALE * (1 + mask_b)) (x_b Wv) @ Wo + bo

Design (transposed-softmax layout; all transposes + dtype casts on host):
  - fp16 x/Wq/Wk/Wv/q/k (better mantissa than bf16, same 1 cyc/row on PE),
    bf16 v/e/out/Wo (e needs bf16 range: exp(t) up to ~e^11); fp32 mask.
  - Scores transposed sT[j, i], two heads per PE pass via row tiling 0/64.
  - DVE does ONE pass over scores: t = sT(PSUM) * m1T -> fp16 SBUF.  This
    is the per-core floor (~1 elem/cyc/lane; fp32 PSUM src locks 1x mode).
  - ACT exponentiates in FD=4096 tiles (4 j-tiles x 2 heads) -> bf16.
  - AV uses ones-augmented V (65th column): denominators fall out of the
    accumulation at PSUM row 64.
  - A pending-work queue (lag 3) defers AV blocks, normalization stages
    and the output projection so the PE FIFO never head-of-line blocks
    on exp (ACT) or reciprocal (DVE).
  - Normalization: av evacuated to SBUF (ACT), reciprocal_approx_fast on
    den rows at partition bases 64/32, K=1 PE outer products broadcast,
    DVE multiplies -> outT (bf16).
  - Q/K projection groups ride the sp PSUM ring, sprinkled through the
    first pair's j-loop; V projections likewise; dummy warm-up matmuls
    open the HAM clock gate before real work arrives.
"""

import numpy as np

B, N, D = 8, 2048, 512
H, DH = 8, 64
SCALE = DH ** -0.5

IC_N, IC = 4, 512      # i-chunks
JT_N, JT = 16, 128     # j-tiles
P_N = 4                # head pairs
QN = 4                 # j-tile quarters per (pair, ic)
LAG = 3                # pending-queue depth (quarter boundaries)

_built = {}


def _build():
    import concourse.tile as tile
    from concourse import bacc, mybir

    F32 = mybir.dt.float32
    F16 = mybir.dt.float16
    BF16 = mybir.dt.bfloat16
    F32R = mybir.dt.float32r
    Exp = mybir.ActivationFunctionType.Exp
    MULT = mybir.AluOpType.mult

    nc = bacc.Bacc()
    xT_d = nc.declare_dram_parameter("xT", [D, N], F16, isOutput=False)
    m1T_d = nc.declare_dram_parameter("m1T", [N, N], F32, isOutput=False)
    wq_d = nc.declare_dram_parameter("wq", [D, D], F16, isOutput=False)
    wk_d = nc.declare_dram_parameter("wk", [D, D], F16, isOutput=False)
    wv_d = nc.declare_dram_parameter("wv", [D, D], F16, isOutput=False)
    wo_d = nc.declare_dram_parameter("wo", [D, D], BF16, isOutput=False)
    bo_d = nc.declare_dram_parameter("bo", [1, D], BF16, isOutput=False)
    y_d = nc.declare_dram_parameter("y", [N, D], F32, isOutput=True)

    with nc.allow_low_precision(reason="fp16/bf16 matmul pipeline"), \
         tile.TileContext(nc) as tc:
        with tc.tile_pool(name="persist", bufs=1) as pp, \
             tc.tile_pool(name="dyn", bufs=1) as p2, \
             tc.tile_pool(name="ps", bufs=1, space="PSUM") as p2p:
            ones_b = pp.tile([128, 128], BF16, tag="ones_b")
            nc.vector.memset(ones_b, 1.0)
            ones_f = pp.tile([128, 64], F32, tag="ones_f")
            nc.vector.memset(ones_f, 1.0)
            ones_r = pp.tile([128, 64], F32R, tag="ones_r")
            nc.vector.tensor_copy(ones_r[:], ones_f[:])

            qT = [pp.tile([128, N], F16, tag=f"qT{t}", name=f"qT{t}") for t in range(4)]
            kT = [pp.tile([128, N], F16, tag=f"kT{t}", name=f"kT{t}") for t in range(4)]
            v_sb = [pp.tile([128, H * 65], BF16, tag=f"v{t}", name=f"v{t}")
                    for t in range(JT_N)]
            wo_sb = [pp.tile([128, D], BF16, tag=f"wo{c}", name=f"wo{c}") for c in range(4)]
            bo_sb = pp.tile([1, D], BF16, tag="bo", name="bo")
            outT = [pp.tile([128, IC], BF16, tag=f"oT{p}", name=f"oT{p}")
                    for p in range(P_N)]

            xT_sb = [pp.tile([128, N], F16, tag=f"xT{c}", name=f"xT{c}") for c in range(4)]
            wq_sb = [pp.tile([128, D], F16, tag=f"wq{c}", name=f"wq{c}") for c in range(4)]
            wk_sb = [pp.tile([128, D], F16, tag=f"wk{c}", name=f"wk{c}") for c in range(4)]
            wv_sb = [pp.tile([128, D], F16, tag=f"wv{c}", name=f"wv{c}") for c in range(4)]

            for c in range(4):
                nc.gpsimd.dma_start(out=wq_sb[c], in_=wq_d[c * 128:(c + 1) * 128, :])
                nc.gpsimd.dma_start(out=wk_sb[c], in_=wk_d[c * 128:(c + 1) * 128, :])
                nc.sync.dma_start(out=xT_sb[c], in_=xT_d[c * 128:(c + 1) * 128, :])
            for c in range(4):
                nc.gpsimd.dma_start(out=wv_sb[c], in_=wv_d[c * 128:(c + 1) * 128, :])
                nc.gpsimd.dma_start(out=wo_sb[c], in_=wo_d[c * 128:(c + 1) * 128, :])
            nc.gpsimd.dma_start(out=bo_sb, in_=bo_d[:])

            # HAM warm-up: ~5us of dummy matmuls opens the PE clock gate.
            warm = p2p.tile([128, 1024], F32, tag="sp", bufs=2, name="warm")
            for _ in range(12):
                nc.tensor.matmul(warm[:, 0:128], ones_b[:], ones_b[:],
                                 start=True, stop=True)

            def qkproj(hdt, which, nch2):
                w_sb, dstT = (wq_sb, qT) if which == 0 else (wk_sb, kT)
                qp = p2p.tile([128, 1024], F32, tag="sp", bufs=2, name="qp")
                for half in range(2):
                    nch = nch2 * 2 + half
                    for c in range(4):
                        nc.tensor.matmul(
                            qp[:, half * 512:(half + 1) * 512],
                            w_sb[c][:, hdt * 128:(hdt + 1) * 128],
                            xT_sb[c][:, nch * 512:(nch + 1) * 512],
                            start=(c == 0), stop=(c == 3))
                nc.scalar.copy(dstT[hdt][:, nch2 * 1024:(nch2 + 1) * 1024], qp[:])

            def vproj(nt):
                vp = p2p.tile([128, 1024], F32, tag="sp", bufs=2, name="vp")
                for c in range(4):
                    nc.tensor.matmul(vp[:, 0:512],
                                     xT_sb[c][:, nt * 128:(nt + 1) * 128],
                                     wv_sb[c][:], start=(c == 0), stop=(c == 3))
                vdst = v_sb[nt].rearrange("p (h e) -> p h e", e=65)
                nc.vector.memset(vdst[:, :, 64:65], 1.0)
                nc.scalar.copy(vdst[:, :, 0:64],
                               vp[:, 0:512].rearrange("p (h e) -> p h e", e=64))

            # head-pair 0 first so attention can start immediately
            for which in (0, 1):
                for nch2 in range(2):
                    qkproj(0, which, nch2)
            # remaining projection groups, sprinkled into pair 0's j-loop
            proj_rest = [(hdt, w, nch2) for hdt in (1, 2, 3)
                         for w in (0, 1) for nch2 in range(2)]

            pending = []

            def push(fn):
                pending.append(fn)
                while len(pending) > LAG:
                    pending.pop(0)()

            for ic in range(IC_N):
                m1T = []
                for jt in range(JT_N):
                    m1t = p2.tile([128, IC], F32, tag="m1T", bufs=JT_N + 2, name="m1T")
                    nc.sync.dma_start(
                        out=m1t,
                        in_=m1T_d[jt * 128:(jt + 1) * 128, ic * IC:(ic + 1) * IC])
                    m1T.append(m1t)

                for p in range(P_N):
                    first = (ic == 0 and p == 0)
                    av0 = p2p.tile([65, 512], F32, tag="av0", bufs=1, name="av0")
                    av1 = p2p.tile([65, 512], F32, tag="av1", bufs=1, name="av1")
                    t4s, e4s = {}, {}

                    def issue_scores(jt, p=p, ic=ic, t4s=t4s, e4s=e4s, m1T=m1T):
                        sp = p2p.tile([128, 1024], F32, tag="sp", bufs=2, name="sp")
                        nc.tensor.matmul(
                            sp[:, 0:512],
                            kT[p][0:64, jt * 128:(jt + 1) * 128],
                            qT[p][0:64, ic * IC:(ic + 1) * IC],
                            start=True, stop=True, tile_position=(0, 0))
                        nc.tensor.matmul(
                            sp[:, 512:1024],
                            kT[p][64:128, jt * 128:(jt + 1) * 128],
                            qT[p][64:128, ic * IC:(ic + 1) * IC],
                            start=True, stop=True, tile_position=(64, 0))
                        q, jtl = divmod(jt, 4)
                        if jtl == 0:
                            t4s[q] = p2.tile([128, 4096], F16, tag="t4",
                                             bufs=3, name="t4")
                        nc.vector.tensor_tensor(
                            out=t4s[q][:, jtl * 1024:(jtl + 1) * 1024]
                                .rearrange("p (h i) -> p h i", h=2),
                            in0=sp[:].rearrange("p (h i) -> p h i", h=2),
                            in1=m1T[jt][:, None, :].broadcast_to((128, 2, IC)),
                            op=MULT)
                        if jtl == 3:
                            e4 = p2.tile([128, 4096], BF16, tag="e4",
                                         bufs=4, name="e4")
                            nc.scalar.activation(e4[:], t4s[q][:], Exp)
                            e4s[q] = e4
                            t4s.pop(q)

                    def make_av(q, p=p, av0=av0, av1=av1, e4s=e4s):
                        e4 = e4s.pop(q)

                        def av_block():
                            for jj in range(4):
                                for h in range(2):
                                    hh = 2 * p + h
                                    nc.tensor.matmul(
                                        (av0 if h == 0 else av1)[:],
                                        v_sb[q * 4 + jj][:, hh * 65:(hh + 1) * 65],
                                        e4[:, (jj * 2 + h) * 512:(jj * 2 + h + 1) * 512],
                                        start=(q == 0 and jj == 0),
                                        stop=(q == QN - 1 and jj == 3))
                        return av_block

                    def make_norms(p=p, av0=av0, av1=av1):
                        tmp0 = p2.tile([64, 512], F32, tag="tmp0", bufs=1, name="tmp0")
                        tmp1 = p2.tile([64, 512], F32, tag="tmp1", bufs=1, name="tmp1")
                        # custom-DVE ops (reciprocal_approx_fast) only work at
                        # partition base 0 -> den rows go through base-0 tiles
                        d0 = p2.tile([1, 512], F32, tag="d0", bufs=1, name="d0")
                        d1 = p2.tile([1, 512], F32, tag="d1", bufs=1, name="d1")
                        rcp0 = p2.tile([1, 512], F32, tag="rcp0", bufs=1, name="rcp0")
                        rcp1 = p2.tile([1, 512], F32, tag="rcp1", bufs=1, name="rcp1")
                        rcp0r = p2.tile([1, 512], F32R, tag="rcp0r", bufs=1,
                                        name="rcp0r")
                        rcp1r = p2.tile([1, 512], F32R, tag="rcp1r", bufs=1,
                                        name="rcp1r")

                        def norm1():
                            nc.scalar.copy(tmp0[:], av0[0:64, :])
                            nc.scalar.copy(tmp1[:], av1[0:64, :])
                            nc.vector.tensor_copy(d0[:], av0[64:65, :])
                            nc.vector.tensor_copy(d1[:], av1[64:65, :])
                            nc.vector.reciprocal_approx_fast(out=rcp0[:], in_=d0[:])
                            nc.vector.reciprocal_approx_fast(out=rcp1[:], in_=d1[:])
                            # fp32 matmuls are broken on HW; bc operands must
                            # be f32r (rounded by the producing copy)
                            nc.scalar.copy(rcp0r[:], rcp0[:])
                            nc.scalar.copy(rcp1r[:], rcp1[:])

                        def norm2():
                            bc0 = p2p.tile([64, 512], F32, tag="bc0", bufs=1, name="bc0")
                            bc1 = p2p.tile([64, 512], F32, tag="bc1", bufs=1, name="bc1")
                            nc.tensor.matmul(bc0[:], ones_r[0:1, :],
                                             rcp0r[:], start=True, stop=True)
                            nc.tensor.matmul(bc1[:], ones_r[0:1, :],
                                             rcp1r[:], start=True, stop=True)
                            nc.vector.tensor_tensor(
                                out=outT[p][0:64, :], in0=tmp0[:], in1=bc0[:],
                                op=MULT)
                            nc.vector.tensor_tensor(
                                out=outT[p][64:128, :], in0=tmp1[:], in1=bc1[:],
                                op=MULT)

                        return norm1, norm2

                    issue_scores(0)
                    issue_scores(1)
                    for jt in range(JT_N):
                        if jt + 2 < JT_N:
                            issue_scores(jt + 2)
                            if first:
                                vproj(jt + 2)
                        if first:
                            if jt < 2:
                                vproj(jt)
                            if jt < len(proj_rest):
                                qkproj(*proj_rest[jt])
                        if jt % 4 == 3:
                            push(make_av(jt // 4))
                            if jt == JT_N - 1:
                                norm1, norm2 = make_norms()
                                push(norm1)
                                push(norm2)

                def make_outproj(ic=ic):
                    def outproj():
                        for itl in range(4):
                            it = ic * 4 + itl
                            ypt = p2p.tile([128, 1024], F32, tag="sp", bufs=2, name="yp")
                            yp = ypt[:, 0:512]
                            nc.tensor.matmul(yp, ones_b[0:1, 0:128], bo_sb[0:1, :],
                                             start=True, stop=False)
                            for hdt in range(4):
                                nc.tensor.matmul(
                                    yp, outT[hdt][:, itl * 128:(itl + 1) * 128],
                                    wo_sb[hdt][:], start=False, stop=(hdt == 3))
                            y_sb = p2.tile([128, D], F32, tag="y_sb", bufs=2, name="y_sb")
                            nc.scalar.copy(y_sb[:], yp)
                            nc.gpsimd.dma_start(out=y_d[it * 128:(it + 1) * 128, :],
                                                in_=y_sb[:])
                    return outproj

                push(make_outproj())

            while pending:
                pending.pop(0)()

    nc.finalize()
    return nc


def _get_nc():
    if "nc" not in _built:
        _built["nc"] = _build()
    return _built["nc"]


def _make_in_maps(inputs):
    import ml_dtypes
    bf16 = ml_dtypes.bfloat16

    x = np.asarray(inputs["x"], dtype=np.float32)
    mask = np.asarray(inputs["mask"], dtype=np.float32)
    wq = (np.asarray(inputs["Wq"], dtype=np.float32) * SCALE).astype(np.float16)
    wk = np.asarray(inputs["Wk"], dtype=np.float32).astype(np.float16)
    wv = np.asarray(inputs["Wv"], dtype=np.float32).astype(np.float16)
    wo = np.asarray(inputs["Wo"], dtype=np.float32).astype(bf16)
    bo2 = np.asarray(inputs["bo"], dtype=np.float32).reshape(1, D).astype(bf16)

    xT = np.ascontiguousarray(x.transpose(0, 2, 1)).astype(np.float16)
    m1T = np.ascontiguousarray((1.0 + mask).transpose(0, 2, 1))

    return [
        {"xT": xT[b], "m1T": m1T[b],
         "wq": wq, "wk": wk, "wv": wv, "wo": wo, "bo": bo2}
        for b in range(B)
    ]


def kernel(x, mask, Wq, Wk, Wv, Wo, bo):
    from concourse.bass_utils import run_bass_kernel_spmd

    nc = _get_nc()
    in_maps = _make_in_maps(dict(x=x, mask=mask, Wq=Wq, Wk=Wk, Wv=Wv, Wo=Wo, bo=bo))
    res = run_bass_kernel_spmd(nc, in_maps, list(range(B)))
    return np.stack([res.results[b]["y"] for b in range(B)], axis=0)
